# revision 11
# baseline (speedup 1.0000x reference)
"""Fused LayerNorm + multi-head attention + output projection on 8 TRN2 NeuronCores.

Sharding: 2-way data parallel over batch x 4-way tensor parallel over heads.
Core c handles batch (c // 4), heads [4*(c%4) .. 4*(c%4)+4).

Device dataflow (everything transposed: host supplies x^T so the feature/
contraction dim always lands on SBUF partitions):
  - LayerNorm is folded into the QKV-projection epilogue:
      qkv^T[n,i] = rstd_i * (raw[n,i] - mu_i * wsum_n) + wb_n
    with raw = W'^T x^T computed on raw x, row stats (mu, rstd) from
    PE ones-matmuls (which broadcast across partitions for free).
  - Scores are computed transposed (S^T[j,i]) so softmax'd probs feed the
    PV matmul without any transpose; two 64-dim heads are packed into the
    128 PE rows via tile_position row groups.
  - Softmax skips max-subtraction (values are bounded; a constant bias in
    the exp cancels in the normalization). The denominator comes from an
    extra ones-column appended to V (M=65 PV matmul).
  - Output projection produces partial^T per core; host sums the 4 TP
    partials per batch, adds b_out, and transposes back.
"""

import os
import sys

import numpy as np

for _p in ("/root/.axon_site", "/root/.axon_site/_ro/trn_rl_repo",
           "/root/.axon_site/_ro/pypackages", "/opt/trn_rl_repo"):
    if os.path.isdir(_p) and _p not in sys.path:
        sys.path.append(_p)

B = 2
N = 2048
DIM = 1024
HEADS = 16
DIM_HEAD = 64
INNER = HEADS * DIM_HEAD
HEADS_PER_CORE = 4          # 4-way tensor parallel on heads
N_CORES = 8
EPS = 1e-5
EXP_BIAS = -4.0             # constant subtracted inside exp; cancels in softmax

KT = DIM // 128             # 8 k-tiles of the contraction dim
IC = 4                      # i-chunks of 512 over N=2048
ICW = N // IC               # 512
JT = N // 128               # 16 j-tiles
NQKV = 3 * HEADS_PER_CORE * DIM_HEAD   # 768 local qkv columns
NT = NQKV // 128            # 6 n-tiles: [q01, q23, k01, k23, v01, v23]
MT = DIM // 128             # 8 output m-tiles

_COMPILED = None


def _build():
    import concourse.bass as bass
    import concourse.mybir as mybir
    from concourse import bacc, tile
    from concourse.masks import make_identity

    f32 = mybir.dt.float32
    bf16 = mybir.dt.bfloat16
    AF = mybir.ActivationFunctionType
    ALU = mybir.AluOpType

    nc = bacc.Bacc("TRN2", target_bir_lowering=False, debug=False,
                   num_devices=N_CORES)

    xT_d = nc.dram_tensor("xT", [DIM, N], bf16, kind="ExternalInput")
    wqkv_d = nc.dram_tensor("wqkv", [DIM, NQKV], bf16, kind="ExternalInput")
    wout_d = nc.dram_tensor("wout", [HEADS_PER_CORE * DIM_HEAD, DIM], bf16,
                            kind="ExternalInput")
    wsum_d = nc.dram_tensor("wsum", [NQKV, 1], f32, kind="ExternalInput")
    wb_d = nc.dram_tensor("wb", [NQKV, 1], f32, kind="ExternalInput")
    out_d = nc.dram_tensor("outT", [DIM, N], f32, kind="ExternalOutput")

    from contextlib import ExitStack

    with ExitStack() as ctx:
        tc = ctx.enter_context(tile.TileContext(nc))
        # persistent pools (whole kernel)
        cst = ctx.enter_context(tc.tile_pool(name="cst", bufs=1))
        qkp = ctx.enter_context(tc.tile_pool(name="qk", bufs=1))
        vaugp = ctx.enter_context(tc.tile_pool(name="vaug", bufs=JT))
        # phase A pools (closed before attention so space is reused)
        actx = ExitStack()
        xp = actx.enter_context(tc.tile_pool(name="xp", bufs=KT))
        wp = actx.enter_context(tc.tile_pool(name="wp", bufs=KT))
        vtp = actx.enter_context(tc.tile_pool(name="vt", bufs=1))
        bcp = actx.enter_context(tc.tile_pool(name="bc", bufs=1))
        scp = actx.enter_context(tc.tile_pool(name="sc", bufs=2))
        ps_stat = actx.enter_context(tc.tile_pool(name="ps_stat", bufs=1, space="PSUM"))
        ps_qkv = actx.enter_context(tc.tile_pool(name="ps_qkv", bufs=3, space="PSUM"))
        ps_tp = actx.enter_context(tc.tile_pool(name="ps_tp", bufs=2, space="PSUM"))
        if True:
            # ---- constants & weight loads ----
            ones = cst.tile([128, 128], bf16)
            nc.vector.memset(ones[:], 1.0)
            eps_t = cst.tile([128, 1], f32, tag="eps")
            nc.vector.memset(eps_t[:], EPS)
            ebias_t = cst.tile([128, 1], f32, tag="ebias")
            nc.vector.memset(ebias_t[:], EXP_BIAS)
            ident = cst.tile([128, 128], bf16)
            make_identity(nc, ident[:])
            wsum_t = cst.tile([128, NT], f32)
            wb_t = cst.tile([128, NT], f32)
            for nt in range(NT):
                nc.sync.dma_start(wsum_t[:, nt:nt + 1],
                                  wsum_d[nt * 128:(nt + 1) * 128, :])
                nc.sync.dma_start(wb_t[:, nt:nt + 1],
                                  wb_d[nt * 128:(nt + 1) * 128, :])

            xt = []
            for k in range(KT):
                t = xp.tile([128, N], bf16)
                nc.sync.dma_start(t[:], xT_d[k * 128:(k + 1) * 128, :])
                xt.append(t)
            wt = []
            for k in range(KT):
                t = wp.tile([128, NQKV], bf16)
                nc.sync.dma_start(t[:], wqkv_d[k * 128:(k + 1) * 128, :])
                wt.append(t)
            wo = []
            for d in range(2):
                t = cst.tile([128, DIM], bf16, tag=f"wo{d}", name=f"wo{d}")
                nc.sync.dma_start(t[:], wout_d[d * 128:(d + 1) * 128, :])
                wo.append(t)

            # ---- phase A.0: LayerNorm row stats (broadcast across partitions
            # via ones-lhsT matmuls: out[m,i] = sum_k x^T[k,i]) ----
            mu_bc = bcp.tile([128, N], f32, tag="mu")
            nrstd_bc = bcp.tile([128, N], f32, tag="nrstd")
            for ic in range(IC):
                isl = slice(ic * ICW, (ic + 1) * ICW)
                sum_ps = ps_stat.tile([128, ICW], f32, tag="sum")
                sq_ps = ps_stat.tile([128, ICW], f32, tag="sq")
                for k in range(KT):
                    x2 = scp.tile([128, ICW], bf16, tag="x2", bufs=3)
                    nc.vector.tensor_mul(x2[:], xt[k][:, isl], xt[k][:, isl])
                    nc.tensor.matmul(sum_ps[:], ones[:], xt[k][:, isl],
                                     start=(k == 0), stop=(k == KT - 1))
                    nc.tensor.matmul(sq_ps[:], ones[:], x2[:],
                                     start=(k == 0), stop=(k == KT - 1))
                # mu = sum/DIM ; var = sumsq/DIM - mu^2 ; nrstd = -1/sqrt(var+eps)
                nc.vector.tensor_scalar_mul(mu_bc[:, isl], sum_ps[:], 1.0 / DIM)
                msq = scp.tile([128, ICW], f32, tag="msq", bufs=1)
                nc.vector.tensor_scalar_mul(msq[:], sq_ps[:], 1.0 / DIM)
                mu2 = scp.tile([128, ICW], f32, tag="mu2", bufs=1)
                nc.vector.tensor_mul(mu2[:], mu_bc[:, isl], mu_bc[:, isl])
                var = scp.tile([128, ICW], f32, tag="var", bufs=1)
                nc.vector.tensor_sub(var[:], msq[:], mu2[:])
                std = scp.tile([128, ICW], f32, tag="std", bufs=1)
                nc.scalar.activation(std[:], var[:], AF.Sqrt, bias=eps_t[:, 0:1])
                rstd = scp.tile([128, ICW], f32, tag="rstd", bufs=1)
                nc.vector.reciprocal(rstd[:], std[:])
                nc.vector.tensor_scalar_mul(nrstd_bc[:, isl], rstd[:], -1.0)

            # ---- phase A.1: QKV projection (transposed outputs) ----
            # n-tile layout: 0,1 -> q^T pairs; 2,3 -> k^T pairs; 4,5 -> v^T
            q01 = qkp.tile([128, N], bf16, tag="q01")
            q23 = qkp.tile([128, N], bf16, tag="q23")
            k01 = qkp.tile([128, N], bf16, tag="k01")
            k23 = qkp.tile([128, N], bf16, tag="k23")
            vT = [vtp.tile([128, N], bf16, tag=f"vt{i}", name=f"vt{i}")
                  for i in range(2)]
            qkv_dst = [q01, q23, k01, k23, vT[0], vT[1]]
            for nt in range(NT):
                nsl = slice(nt * 128, (nt + 1) * 128)
                for ic in range(IC):
                    isl = slice(ic * ICW, (ic + 1) * ICW)
                    ps = ps_qkv.tile([128, ICW], f32, tag="qkv")
                    for k in range(KT):
                        nc.tensor.matmul(ps[:], wt[k][:, nsl],
                                         xt[k][:, isl],
                                         start=(k == 0), stop=(k == KT - 1))
                    # (mu*wsum - raw) * (-rstd) + wb
                    tmp = scp.tile([128, ICW], f32, tag="fix", bufs=3)
                    nc.vector.scalar_tensor_tensor(
                        tmp[:], mu_bc[:, isl], wsum_t[:, nt:nt + 1], ps[:],
                        op0=ALU.mult, op1=ALU.subtract)
                    dst = qkv_dst[nt][:, isl]
                    nc.vector.tensor_mul(dst, tmp[:], nrstd_bc[:, isl])
                    nc.vector.tensor_scalar_add(dst, dst, wb_t[:, nt:nt + 1])

            # ---- phase A.2: transpose v^T -> v_aug[j,d] with ones column ----
            # v_aug layout per j-tile: head h occupies cols [65h, 65h+64) and
            # col 65h+64 is 1.0 (softmax denominator via M=65 PV matmul).
            vaug = []
            for j in range(JT):
                va = vaugp.tile([128, 4 * 65], bf16, tag="vaug")
                ones_col = va[:, 64:4 * 65:65]
                nc.vector.memset(ones_col, 1.0)
                for d in range(2):
                    tp = ps_tp.tile([128, 128], bf16, tag="tp")
                    nc.tensor.transpose(tp[:], vT[d][:, j * 128:(j + 1) * 128],
                                        ident[:])
                    h0 = 2 * d
                    nc.vector.tensor_copy(va[:, 65 * h0:65 * h0 + 64],
                                          tp[:, 0:64])
                    nc.vector.tensor_copy(va[:, 65 * (h0 + 1):65 * (h0 + 1) + 64],
                                          tp[:, 64:128])
                vaug.append(va)

            # ---- phase B: attention + output projection, per i-chunk ----
            actx.close()
            ep = ctx.enter_context(tc.tile_pool(name="ep", bufs=3))
            onp = ctx.enter_context(tc.tile_pool(name="on", bufs=4))
            otp = ctx.enter_context(tc.tile_pool(name="ot", bufs=3))
            smp = ctx.enter_context(tc.tile_pool(name="sm", bufs=2))
            ps_s = ctx.enter_context(tc.tile_pool(name="ps_s", bufs=2, space="PSUM"))
            ps_o = ctx.enter_context(tc.tile_pool(name="ps_o", bufs=2, space="PSUM"))
            ps_pj = ctx.enter_context(tc.tile_pool(name="ps_pj", bufs=2, space="PSUM"))
            qt_pair = [q01, q23]
            kt_pair = [k01, k23]
            for ic in range(IC):
                isl = slice(ic * ICW, (ic + 1) * ICW)
                o_norm = [onp.tile([128, ICW], bf16, tag="onorm", name="onorm")
                          for _ in range(2)]
                for pair in range(2):
                    qt = qt_pair[pair]
                    kt = kt_pair[pair]
                    o_ps = [ps_o.tile([65, ICW], f32, tag="o", name="o_ps")
                            for _ in range(2)]
                    for j in range(JT):
                        jsl = slice(j * 128, (j + 1) * 128)
                        s_ps = ps_s.tile([128, 2 * ICW], f32, tag="s")
                        e_t = ep.tile([128, 2 * ICW], bf16, tag="e")
                        for hh in range(2):
                            psl = slice(hh * 64, (hh + 1) * 64)
                            nc.tensor.matmul(s_ps[:, hh * ICW:(hh + 1) * ICW],
                                             kt[psl, jsl], qt[psl, isl])
                        nc.scalar.activation(e_t[:], s_ps[:], AF.Exp,
                                             bias=ebias_t[:, 0:1])
                        for hh in range(2):
                            h = 2 * pair + hh
                            nc.tensor.matmul(
                                o_ps[hh][:],
                                vaug[j][:, 65 * h:65 * h + 65],
                                e_t[:, hh * ICW:(hh + 1) * ICW],
                                start=(j == 0), stop=(j == JT - 1))
                    # normalize: O[d,i] / l_i  (l is row 64 of o_ps)
                    for hh in range(2):
                        o_sb = smp.tile([65, ICW], f32, tag="osb")
                        nc.vector.tensor_copy(o_sb[:], o_ps[hh][:])
                        lrow = smp.tile([1, ICW], f32, tag="lrow")
                        nc.sync.dma_start(lrow[:], o_sb[64:65, :])
                        linv = smp.tile([1, ICW], f32, tag="linv")
                        nc.vector.reciprocal(linv[:], lrow[:])
                        lbc = smp.tile([64, ICW], f32, tag="lbc")
                        nc.gpsimd.partition_broadcast(lbc[:], linv[:])
                        if hh == 0:
                            nc.vector.tensor_mul(o_norm[pair][0:64, :],
                                                 o_sb[0:64, :], lbc[:])
                        else:
                            ob = smp.tile([64, ICW], bf16, tag="ob")
                            nc.vector.tensor_mul(ob[:], o_sb[0:64, :],
                                                 lbc[:])
                            nc.sync.dma_start(o_norm[pair][64:128, :], ob[:])
                # output projection for this i-chunk
                for mt in range(MT):
                    msl = slice(mt * 128, (mt + 1) * 128)
                    pps = ps_pj.tile([128, ICW], f32, tag="pj")
                    for d in range(2):
                        nc.tensor.matmul(pps[:], wo[d][:, msl],
                                         o_norm[d][:],
                                         start=(d == 0), stop=(d == 1))
                    ot = otp.tile([128, ICW], f32, tag="ot")
                    nc.vector.tensor_copy(ot[:], pps[:])
                    nc.sync.dma_start(out_d[msl, isl], ot[:])

    nc.compile()
    return nc


def _get_compiled():
    global _COMPILED
    if _COMPILED is None:
        _COMPILED = _build()
    return _COMPILED


def _shard_inputs(x, ln_gamma, ln_beta, w_qkv, w_out):
    """Build per-core input maps (all host-side layout work, no math on x)."""
    x = np.ascontiguousarray(np.asarray(x, np.float32))
    g = np.asarray(ln_gamma, np.float32)
    be = np.asarray(ln_beta, np.float32)
    wq = np.asarray(w_qkv, np.float32)
    wo = np.asarray(w_out, np.float32)

    scale = DIM_HEAD ** (-0.5)
    wq_g = wq * g[:, None]            # gamma folded
    wq_g[:, :INNER] *= scale          # attention scale folded into W_q
    wb_full = be @ wq                 # beta contribution
    wb_full[:INNER] *= scale

    in_maps = []
    for c in range(N_CORES):
        b = c // HEADS_PER_CORE
        grp = c % HEADS_PER_CORE
        heads = [HEADS_PER_CORE * grp + t for t in range(HEADS_PER_CORE)]
        cols = []
        for which in range(3):        # q, k, v column blocks
            for h in heads:
                lo = which * INNER + h * DIM_HEAD
                cols.append(np.arange(lo, lo + DIM_HEAD))
        cols = np.concatenate(cols)
        wqkv_c = np.ascontiguousarray(wq_g[:, cols])
        wsum_c = wqkv_c.sum(axis=0).reshape(NQKV, 1)
        wb_c = wb_full[cols].reshape(NQKV, 1)
        rows = np.concatenate([np.arange(h * DIM_HEAD, (h + 1) * DIM_HEAD)
                               for h in heads])
        wout_c = np.ascontiguousarray(wo[rows, :])
        import ml_dtypes
        bf = ml_dtypes.bfloat16
        in_maps.append({
            "xT": np.ascontiguousarray(x[b].T).astype(bf),
            "wqkv": wqkv_c.astype(bf),
            "wout": wout_c.astype(bf),
            "wsum": np.ascontiguousarray(wsum_c),
            "wb": np.ascontiguousarray(wb_c),
        })
    return in_maps


def _run(inputs, trace=False):
    from concourse.bass_utils import run_bass_kernel_spmd

    nc = _get_compiled()
    in_maps = _shard_inputs(inputs["x"], inputs["ln_gamma"],
                            inputs["ln_beta"], inputs["w_qkv"],
                            inputs["w_out"])
    res = run_bass_kernel_spmd(nc, in_maps, core_ids=list(range(N_CORES)),
                               trace=trace)
    b_out = np.asarray(inputs["b_out"], np.float32)
    outs = []
    for b in range(B):
        acc = np.zeros((DIM, N), np.float32)
        for grp in range(HEADS_PER_CORE):
            acc += res.results[b * HEADS_PER_CORE + grp]["outT"]
        outs.append(acc.T + b_out)
    out = np.stack(outs).astype(np.float32)
    return out, res


def kernel(**inputs):
    out, _ = _run(inputs, trace=False)
    return out


# revision 14
# speedup vs baseline: 1.2620x; 1.2620x over previous
"""Fused LayerNorm + multi-head attention + output projection on 8 TRN2 NeuronCores.

Sharding: 2-way data parallel over batch x 4-way tensor parallel over heads.
Core c handles batch (c // 4), heads [4*(c%4) .. 4*(c%4)+4).

Device dataflow (everything transposed: host supplies x^T so the feature/
contraction dim always lands on SBUF partitions):
  - LayerNorm is folded into the QKV-projection epilogue:
      qkv^T[n,i] = rstd_i * (raw[n,i] - mu_i * wsum_n) (+ wb_n)
    with raw = W'^T x^T computed on raw x, row stats (mu, rstd) from
    PE ones-matmuls (which broadcast across partitions for free).
  - Scores are computed transposed (S^T[j,i]) so softmax'd probs feed the
    PV matmul without any transpose; two 64-dim heads are packed into the
    128 PE rows via tile_position row groups.
  - Softmax skips max-subtraction (values are bounded; a constant bias in
    the exp cancels in the normalization). The denominator comes from an
    extra ones-column appended to V (M=65 PV matmul).
  - The exp stream on the Scalar engine is the critical path, so the
    second head-group's QKV projection + V transposes are emitted
    interleaved into head-group 0's attention loop to overlap on the PE.
  - Output projection produces partial^T per core; host sums the 4 TP
    partials per batch, adds b_out, and transposes back.
"""

import os
import sys
from collections import deque

import numpy as np

for _p in ("/root/.axon_site", "/root/.axon_site/_ro/trn_rl_repo",
           "/root/.axon_site/_ro/pypackages", "/opt/trn_rl_repo"):
    if os.path.isdir(_p) and _p not in sys.path:
        sys.path.append(_p)

B = 2
N = 2048
DIM = 1024
HEADS = 16
DIM_HEAD = 64
INNER = HEADS * DIM_HEAD
HEADS_PER_CORE = 4          # 4-way tensor parallel on heads
N_CORES = 8
EPS = 1e-5
EXP_BIAS = -4.0             # constant subtracted inside exp; cancels in softmax

KT = DIM // 128             # 8 k-tiles of the contraction dim
IC = 4                      # i-chunks of 512 over N=2048
ICW = N // IC               # 512
JT = N // 128               # 16 j-tiles
NQKV = 3 * HEADS_PER_CORE * DIM_HEAD   # 768 local qkv columns
NT = NQKV // 128            # 6 n-tiles: [q01, q23, k01, k23, v01, v23]
MT = DIM // 128             # 8 output m-tiles

_COMPILED = {}
INTERLEAVE_BG = True


def _build(has_wb):
    import concourse.bass as bass
    import concourse.mybir as mybir
    from concourse import bacc, tile
    from concourse.masks import make_identity
    from contextlib import ExitStack

    f32 = mybir.dt.float32
    bf16 = mybir.dt.bfloat16
    AF = mybir.ActivationFunctionType
    ALU = mybir.AluOpType

    nc = bacc.Bacc("TRN2", target_bir_lowering=False, debug=False,
                   num_devices=N_CORES)

    xT_d = nc.dram_tensor("xT", [DIM, N], bf16, kind="ExternalInput")
    wqkv_d = nc.dram_tensor("wqkv", [DIM, NQKV], bf16, kind="ExternalInput")
    wout_d = nc.dram_tensor("wout", [HEADS_PER_CORE * DIM_HEAD, DIM], bf16,
                            kind="ExternalInput")
    wsum_d = nc.dram_tensor("wsum", [NQKV, 1], f32, kind="ExternalInput")
    wb_d = nc.dram_tensor("wb", [NQKV, 1], f32, kind="ExternalInput")
    out_d = nc.dram_tensor("outT", [DIM, N], f32, kind="ExternalOutput")

    with ExitStack() as ctx:
        tc = ctx.enter_context(tile.TileContext(nc))
        cst = ctx.enter_context(tc.tile_pool(name="cst", bufs=1))
        xp = ctx.enter_context(tc.tile_pool(name="xp", bufs=KT))
        wp = ctx.enter_context(tc.tile_pool(name="wp", bufs=KT))
        qkp = ctx.enter_context(tc.tile_pool(name="qk", bufs=1))
        vtp = ctx.enter_context(tc.tile_pool(name="vt", bufs=1))
        vaugp = ctx.enter_context(tc.tile_pool(name="vaug", bufs=JT))
        bcp = ctx.enter_context(tc.tile_pool(name="bc", bufs=1))
        scp = ctx.enter_context(tc.tile_pool(name="sc", bufs=2))
        ep = ctx.enter_context(tc.tile_pool(name="ep", bufs=3))
        onp = ctx.enter_context(tc.tile_pool(name="on", bufs=2 * IC))
        otp = ctx.enter_context(tc.tile_pool(name="ot", bufs=3))
        smp = ctx.enter_context(tc.tile_pool(name="sm", bufs=2))
        # single PSUM pool, 8 banks total:
        #   tag "s": 2 x [128,1024] (2 banks each) -> 4 banks
        #            (also reused for the LN stats accumulators up front)
        #   tag "o": 2 x [65,512]                  -> 2 banks
        #   tag "g": 2 x [128,512]                 -> 2 banks
        #            (qkv groups, v transposes, out projection)
        psum = ctx.enter_context(tc.tile_pool(name="psum", bufs=2,
                                              space="PSUM"))

        # ---- constants & weight loads ----
        ones = cst.tile([128, 128], bf16)
        nc.vector.memset(ones[:], 1.0)
        eps_t = cst.tile([128, 1], f32, tag="eps")
        nc.vector.memset(eps_t[:], EPS)
        ebias_t = cst.tile([128, 1], f32, tag="ebias")
        nc.vector.memset(ebias_t[:], EXP_BIAS)
        ident = cst.tile([128, 128], bf16)
        make_identity(nc, ident[:])
        wsum_t = cst.tile([128, NT], f32)
        wb_t = cst.tile([128, NT], f32)
        for nt in range(NT):
            nc.sync.dma_start(wsum_t[:, nt:nt + 1],
                              wsum_d[nt * 128:(nt + 1) * 128, :])
            if has_wb:
                nc.sync.dma_start(wb_t[:, nt:nt + 1],
                                  wb_d[nt * 128:(nt + 1) * 128, :])

        xt = []
        for k in range(KT):
            t = xp.tile([128, N], bf16, tag="xt", name=f"xt{k}")
            nc.sync.dma_start(t[:], xT_d[k * 128:(k + 1) * 128, :])
            xt.append(t)
        wt = []
        for k in range(KT):
            t = wp.tile([128, NQKV], bf16, tag="wt", name=f"wt{k}")
            nc.sync.dma_start(t[:], wqkv_d[k * 128:(k + 1) * 128, :])
            wt.append(t)
        wo = []
        for d in range(2):
            t = cst.tile([128, DIM], bf16, tag=f"wo{d}", name=f"wo{d}")
            nc.sync.dma_start(t[:], wout_d[d * 128:(d + 1) * 128, :])
            wo.append(t)

        # ---- LayerNorm row stats ----
        mu_bc = bcp.tile([128, N], f32, tag="mu")
        nrstd_bc = bcp.tile([128, N], f32, tag="nrstd")
        for ic in range(IC):
            isl = slice(ic * ICW, (ic + 1) * ICW)
            st_ps = psum.tile([128, 2 * ICW], f32, tag="s", name="st_ps")
            for k in range(KT):
                x2 = scp.tile([128, ICW], bf16, tag="x2", bufs=3, name="x2")
                nc.vector.tensor_mul(x2[:], xt[k][:, isl], xt[k][:, isl])
                nc.tensor.matmul(st_ps[:, 0:ICW], ones[:], xt[k][:, isl],
                                 start=(k == 0), stop=(k == KT - 1))
                nc.tensor.matmul(st_ps[:, ICW:2 * ICW], ones[:], x2[:],
                                 start=(k == 0), stop=(k == KT - 1))
            # mu = sum/DIM ; var = sumsq/DIM - mu^2 ; nrstd = -1/sqrt(var+eps)
            nc.vector.tensor_scalar_mul(mu_bc[:, isl], st_ps[:, 0:ICW],
                                        1.0 / DIM)
            msq = scp.tile([128, ICW], f32, tag="msq", bufs=1, name="msq")
            nc.vector.tensor_scalar_mul(msq[:], st_ps[:, ICW:2 * ICW],
                                        1.0 / DIM)
            mu2 = scp.tile([128, ICW], f32, tag="mu2", bufs=1, name="mu2")
            nc.vector.tensor_mul(mu2[:], mu_bc[:, isl], mu_bc[:, isl])
            var = scp.tile([128, ICW], f32, tag="var", bufs=1, name="var")
            nc.vector.tensor_sub(var[:], msq[:], mu2[:])
            std = scp.tile([128, ICW], f32, tag="std", bufs=1, name="std")
            nc.scalar.activation(std[:], var[:], AF.Sqrt, bias=eps_t[:, 0:1])
            rstd = scp.tile([128, ICW], f32, tag="rstd", bufs=1, name="rstd")
            rsc = scp.tile([128, ICW], f32, tag="rsc", bufs=1, name="rsc")
            nc.vector.reciprocal_approx_accurate(rstd[:], std[:], rsc[:])
            nc.vector.tensor_scalar_mul(nrstd_bc[:, isl], rstd[:], -1.0)

        # ---- QKV projection (transposed outputs) ----
        q01 = qkp.tile([128, N], bf16, tag="q01")
        q23 = qkp.tile([128, N], bf16, tag="q23")
        k01 = qkp.tile([128, N], bf16, tag="k01")
        k23 = qkp.tile([128, N], bf16, tag="k23")
        vT = [vtp.tile([128, N], bf16, tag=f"vt{i}", name=f"vt{i}")
              for i in range(2)]
        qkv_dst = [q01, q23, k01, k23, vT[0], vT[1]]

        def qkv_group(nt, ic):
            nsl = slice(nt * 128, (nt + 1) * 128)
            isl = slice(ic * ICW, (ic + 1) * ICW)
            ps = psum.tile([128, ICW], f32, tag="g", name="qkv_ps")
            for k in range(KT):
                nc.tensor.matmul(ps[:], wt[k][:, nsl], xt[k][:, isl],
                                 start=(k == 0), stop=(k == KT - 1))
            # (mu*wsum - raw) * (-rstd) [+ wb]
            tmp = scp.tile([128, ICW], f32, tag="fix", bufs=3, name="fix")
            nc.vector.scalar_tensor_tensor(
                tmp[:], mu_bc[:, isl], wsum_t[:, nt:nt + 1], ps[:],
                op0=ALU.mult, op1=ALU.subtract)
            dst = qkv_dst[nt][:, isl]
            nc.vector.tensor_mul(dst, tmp[:], nrstd_bc[:, isl])
            if has_wb:
                nc.vector.tensor_scalar_add(dst, dst, wb_t[:, nt:nt + 1])

        # v_aug layout per (pair, j-tile): local head hh occupies cols
        # [65hh, 65hh+64) and col 65hh+64 is 1.0 (softmax denominator via
        # the M=65 PV matmul). Separate tiles per pair so the background
        # transposes for pair 1 never touch tiles pair 0 is reading.
        vaug = [[None] * JT for _ in range(2)]

        def v_transpose(d, j):
            va = vaugp.tile([128, 2 * 65], bf16, tag=f"vaug{d}",
                            name=f"vaug{d}_{j}", bufs=JT)
            vaug[d][j] = va
            nc.vector.memset(va[:, 64:2 * 65:65], 1.0)
            tp = psum.tile([128, 128], bf16, tag="g", name="tp")
            nc.tensor.transpose(tp[:], vT[d][:, j * 128:(j + 1) * 128],
                                ident[:])
            nc.vector.tensor_copy(va[:, 0:64], tp[:, 0:64])
            nc.vector.tensor_copy(va[:, 65:129], tp[:, 64:128])

        # part 1 (needed before pair-0 attention): q01, k01, v01 + transposes
        for nt in (0, 2, 4):
            for ic in range(IC):
                qkv_group(nt, ic)
        for j in range(JT):
            v_transpose(0, j)

        # part 2 is emitted interleaved into pair-0's attention loop so its
        # PE work overlaps the ACT-bound exp stream.
        bg = deque()
        for nt in (1, 3, 5):
            for ic in range(IC):
                bg.append((qkv_group, nt, ic))
        for j in range(JT):
            bg.append((v_transpose, 1, j))
        if not INTERLEAVE_BG:
            while bg:
                fn, a, b2 = bg.popleft()
                fn(a, b2)

        # ---- attention + output projection ----
        qt_pair = [q01, q23]
        kt_pair = [k01, k23]
        o_norm = [[onp.tile([128, ICW], bf16, tag="onorm",
                            name=f"onorm{p}_{i}") for i in range(IC)]
                  for p in range(2)]
        for pair in range(2):
            qt = qt_pair[pair]
            kt = kt_pair[pair]
            for ic in range(IC):
                isl = slice(ic * ICW, (ic + 1) * ICW)
                o_ps = [psum.tile([65, ICW], f32, tag="o", name="o_ps")
                        for _ in range(2)]
                for j in range(JT):
                    jsl = slice(j * 128, (j + 1) * 128)
                    s_ps = psum.tile([128, 2 * ICW], f32, tag="s", name="s_ps")
                    e_t = ep.tile([128, 2 * ICW], bf16, tag="e", name="e_t")
                    for hh in range(2):
                        psl = slice(hh * 64, (hh + 1) * 64)
                        nc.tensor.matmul(s_ps[:, hh * ICW:(hh + 1) * ICW],
                                         kt[psl, jsl], qt[psl, isl])
                    nc.scalar.activation(e_t[:], s_ps[:], AF.Exp,
                                         bias=ebias_t[:, 0:1])
                    for hh in range(2):
                        nc.tensor.matmul(
                            o_ps[hh][:],
                            vaug[pair][j][:, 65 * hh:65 * hh + 65],
                            e_t[:, hh * ICW:(hh + 1) * ICW],
                            start=(j == 0), stop=(j == JT - 1))
                    if pair == 0 and bg and INTERLEAVE_BG:
                        fn, a, b2 = bg.popleft()
                        fn(a, b2)
                # normalize: O[d,i] / l_i  (l is row 64 of o_ps)
                for hh in range(2):
                    o_sb = smp.tile([65, ICW], f32, tag="osb", name="osb")
                    nc.vector.tensor_copy(o_sb[:], o_ps[hh][:])
                    lrow = smp.tile([1, ICW], f32, tag="lrow", name="lrow")
                    nc.sync.dma_start(lrow[:], o_sb[64:65, :])
                    linv = smp.tile([1, ICW], f32, tag="linv", name="linv")
                    lsc = smp.tile([1, ICW], f32, tag="lsc", name="lsc")
                    nc.vector.reciprocal_approx_accurate(linv[:], lrow[:],
                                                         lsc[:])
                    lbc = smp.tile([64, ICW], f32, tag="lbc", name="lbc")
                    nc.gpsimd.partition_broadcast(lbc[:], linv[:])
                    if hh == 0:
                        nc.vector.tensor_mul(o_norm[pair][ic][0:64, :],
                                             o_sb[0:64, :], lbc[:])
                    else:
                        ob = smp.tile([64, ICW], bf16, tag="ob", name="ob")
                        nc.vector.tensor_mul(ob[:], o_sb[0:64, :], lbc[:])
                        nc.sync.dma_start(o_norm[pair][ic][64:128, :], ob[:])
                # output projection for this i-chunk (after both pairs done)
                if pair == 1:
                    for mt in range(MT):
                        msl = slice(mt * 128, (mt + 1) * 128)
                        pps = psum.tile([128, ICW], f32, tag="g", name="pj_ps")
                        for d in range(2):
                            nc.tensor.matmul(pps[:], wo[d][:, msl],
                                             o_norm[d][ic][:],
                                             start=(d == 0), stop=(d == 1))
                        ot = otp.tile([128, ICW], f32, tag="ot", name="ot")
                        nc.vector.tensor_copy(ot[:], pps[:])
                        nc.sync.dma_start(out_d[msl, isl], ot[:])

    nc.compile()
    return nc


def _get_compiled(has_wb):
    key = bool(has_wb)
    if key not in _COMPILED:
        _COMPILED[key] = _build(key)
    return _COMPILED[key]


def _shard_inputs(x, ln_gamma, ln_beta, w_qkv, w_out):
    """Build per-core input maps (all host-side layout work, no math on x)."""
    import ml_dtypes
    bf = ml_dtypes.bfloat16

    x = np.ascontiguousarray(np.asarray(x, np.float32))
    g = np.asarray(ln_gamma, np.float32)
    be = np.asarray(ln_beta, np.float32)
    wq = np.asarray(w_qkv, np.float32)
    wo = np.asarray(w_out, np.float32)

    scale = DIM_HEAD ** (-0.5)
    wq_g = wq * g[:, None]            # gamma folded
    wq_g[:, :INNER] *= scale          # attention scale folded into W_q
    wb_full = be @ wq                 # beta contribution
    wb_full[:INNER] *= scale

    in_maps = []
    for c in range(N_CORES):
        b = c // HEADS_PER_CORE
        grp = c % HEADS_PER_CORE
        heads = [HEADS_PER_CORE * grp + t for t in range(HEADS_PER_CORE)]
        # column order: [q01, q23, k01, k23, v01, v23] pair-tiles
        cols = []
        for which in range(3):        # q, k, v
            for h in heads:
                lo = which * INNER + h * DIM_HEAD
                cols.append(np.arange(lo, lo + DIM_HEAD))
        cols = np.concatenate(cols)
        wqkv_c = np.ascontiguousarray(wq_g[:, cols])
        # bf16-round the weights before computing wsum so the LN-fold
        # correction matches what the device matmul actually sums.
        wqkv_bf = wqkv_c.astype(bf)
        wsum_c = wqkv_bf.astype(np.float32).sum(axis=0).reshape(NQKV, 1)
        wb_c = wb_full[cols].reshape(NQKV, 1)
        rows = np.concatenate([np.arange(h * DIM_HEAD, (h + 1) * DIM_HEAD)
                               for h in heads])
        wout_c = np.ascontiguousarray(wo[rows, :])
        in_maps.append({
            "xT": np.ascontiguousarray(x[b].T).astype(bf),
            "wqkv": wqkv_bf,
            "wout": wout_c.astype(bf),
            "wsum": np.ascontiguousarray(wsum_c),
            "wb": np.ascontiguousarray(wb_c),
        })
    return in_maps


def _run(inputs, trace=False):
    from concourse.bass_utils import run_bass_kernel_spmd

    in_maps = _shard_inputs(inputs["x"], inputs["ln_gamma"],
                            inputs["ln_beta"], inputs["w_qkv"],
                            inputs["w_out"])
    has_wb = bool(np.max(np.abs(in_maps[0]["wb"])) > 0)
    nc = _get_compiled(has_wb)
    res = run_bass_kernel_spmd(nc, in_maps, core_ids=list(range(N_CORES)),
                               trace=trace)
    b_out = np.asarray(inputs["b_out"], np.float32)
    outs = []
    for b in range(B):
        acc = np.zeros((DIM, N), np.float32)
        for grp in range(HEADS_PER_CORE):
            acc += res.results[b * HEADS_PER_CORE + grp]["outT"]
        outs.append(acc.T + b_out)
    out = np.stack(outs).astype(np.float32)
    return out, res


def kernel(**inputs):
    out, _ = _run(inputs, trace=False)
    return out
